# revision 6
# baseline (speedup 1.0000x reference)
"""DilatedAttention Trainium2 kernel (8-core SPMD, Bass/Tile), v3.

Reference computation (B=4, L=8192, D=768, SEG=2048, RATE=4):
  q/k/v = sparsify(Q/K/V)            # every RATE-th row per segment -> [B,2048,768]
  q,k,v = x @ W{q,k,v}.T             # torch Linear, no bias
  q,k   = LayerNorm(q/k) * gamma + beta
  attn  = softmax(q @ k.T / sqrt(768))
  out   = softmax(attn @ v, axis=-1)  # final softmax over features

Sharding: core c handles batch b=c//2, query-half h=c%2 (1024 queries)
AND key-half h (1024 keys).  K/V projections are NOT duplicated: each
core projects only its key-half, then the pair exchanges k_ln/v_pr
(fp8, ~0.8MB each) via pairwise AllGathers through DRAM bounce buffers
(outputs in Shared scratchpad).  The gathers overlap the q projection.

Scheduling: all raw input DMAs issue first on the sync ring; weight
DMAs stream as six fat contiguous chunks each on the scalar ring; the
bounce writes + collective triggers live on the gpsimd ring (doorbells
are non-blocking); the gathered reads + output DMAs ride sync.  LN
finish multiplies split 3/3 across gpsimd and DVE; v PSUM evacuations
split across DVE casts and scalar ACT-Copy(scale=1/16).

On-device phases (feature-major):
  1. k-proj (bf16) -> LN -> k_stg fp8 -> bounce -> AllGather -> k_full.
  2. v-proj (fp8 DoubleRow) -> v_stg [m,776] fp8 (ones cols for the
     in-matmul sumexp) -> AllGather -> v_full.
  3. q-proj (bf16) -> LN -> q_ln fp8 (overlaps both gathers).
  4. scores: fp8 DR matmuls, 2 key-chunks per 2-bank PSUM tile, ONE exp
     ACT per pair (halves ACT instruction overhead).
  5. attnV fp8 DR with sumexp riding in PSUM col 768; final softmax via
     exp(scale=1/sumexp) + accum, normalize on DVE, f16 output DMA.
"""

import numpy as np

import concourse.bass as bass
import concourse.tile as tile
from concourse import bacc, mybir
from concourse.bass_utils import run_bass_kernel_spmd

F32 = mybir.dt.float32
F16 = mybir.dt.float16
BF16 = mybir.dt.bfloat16
F8 = mybir.dt.float8e4
AF = mybir.ActivationFunctionType
DR = mybir.MatmulPerfMode.DoubleRow

SEG, RATE, D, B, L = 2048, 4, 768, 4, 8192
LS = (L // SEG) * (SEG // RATE)  # 2048 sparsified tokens per batch
MQ = LS // 2                     # 1024 queries per core
MH = LS // 2                     # 1024 local keys per core
DC = D // 128                    # 6 feature chunks
KT = LS // 128                   # 16 key-token chunks (full)
KH = MH // 128                   # 8 local key-token chunks
LN_EPS = 1e-5
SCALE = 1.0 / float(np.sqrt(D))
DV = D + 16                      # v row width: 768 + ones cols
EXP_BIAS = -0.5                  # PT = exp(z-0.5): keeps e4m3 PT under 240

N_CORES = 8
PAIRS = [[0, 1], [2, 3], [4, 5], [6, 7]]
BLK = 512
MQQ = 512
NQ = MQ // MQQ


def _emit(tc, ins, out, apply_gb):
    nc = tc.nc
    qt, kt, vt, wq, wk, wv, gm, bt = ins

    pools = {}

    def pool(name, bufs, **kw):
        if name not in pools:
            pools[name] = tc.alloc_tile_pool(name=name, bufs=bufs, **kw)
        return pools[name]

    sing = pool("sing", 1)
    wpool = pool("w", 3)
    raw = pool("raw", 4)        # bf16 raw blocks (kt+qt ring)
    big = pool("big", 1)        # persistent tensors
    stg = pool("stg", 3)        # bf16 proj staging; also rstd rows
    ptp = pool("ptp", 2)
    sq = pool("sq", 6)
    rbc = stg
    fin = pool("fin", 3)
    scal = fin
    dram = pool("dram", 1, space="DRAM")
    ps1 = tc.alloc_tile_pool(name="ps1", bufs=2, space="PSUM")

    ones8 = sing.tile([128, 2, 128], F8)
    nc.vector.memset(ones8, 1.0)
    eps_t = sing.tile([128, 1], F32)
    nc.vector.memset(eps_t, LN_EPS)
    ebias_t = sing.tile([128, 1], F32)
    nc.vector.memset(ebias_t, EXP_BIAS)
    if apply_gb:
        gm_sb = sing.tile([128, DC], F32)
        nc.sync.dma_start(gm_sb, gm.rearrange("(c p) -> p c", p=128))
        bt_sb = sing.tile([128, DC], F32)
        nc.sync.dma_start(bt_sb, bt.rearrange("(c p) -> p c", p=128))

    # ---- input + weight DMAs all issue up front ----
    # raw inputs on the sync ring (k first: its projection leads)
    k_rbs, v_rbs, q_rbs = [], [], []
    for mb in range(MH // BLK):
        rb = raw.tile([128, DC, BLK], BF16, tag="raw", name=f"krb{mb}")
        nc.sync.dma_start(rb, kt[:, mb])
        k_rbs.append(rb)
    for mb in range(MH // BLK):
        rb = raw.tile([128, DC, BLK], F8, tag="rawv", name=f"vrb{mb}")
        nc.sync.dma_start(rb, vt[:, mb])
        v_rbs.append(rb)
    for mb in range(MQ // BLK):
        rb = raw.tile([128, DC, BLK], BF16, tag="raw", name=f"qrb{mb}")
        nc.sync.dma_start(rb, qt[:, mb])
        q_rbs.append(rb)

    # weights stream in six fat chunks each on the scalar ring:
    # SBUF layout [128, DC_out, DC_in, 128] keeps every chunk DMA one
    # contiguous 1536B run per partition
    def load_w6(wdram):
        t = wpool.tile([128, DC, DC, 128], BF16, tag="w")
        for j in range(DC):
            nc.scalar.dma_start(t[:, j], wdram[j])
        return t

    wk_t = load_w6(wk)
    wv_t = wpool.tile([128, DC, D], F8, tag="w")
    nc.scalar.dma_start(wv_t, wv)
    wq_t = load_w6(wq)

    # PE warmup while the first DMAs land
    wu_l = sing.tile([128, 128], BF16)
    nc.vector.memset(wu_l, 0.0)
    wu_r = sing.tile([128, 512], BF16)
    nc.vector.memset(wu_r, 0.0)
    psum_w = ps1.tile([128, DV], F32, tag="acc")
    for _ in range(36):
        nc.tensor.matmul(psum_w[:, 0:512], wu_l, wu_r, start=True, stop=True)
    wu_g = sing.tile([1, 8], F32)
    nc.vector.tensor_copy(wu_g, psum_w[0:1, 0:8])

    # persistent tensors
    q_ln = big.tile([128, DC, MQ], F8, tag="q_ln")
    k_stg = big.tile([128, DC, MH], F8, tag="k_stg")
    k_full = big.tile([128, DC, LS], F8, tag="k_full")
    v_stg = big.tile([128, KH, DV], F8, tag="v_stg")
    v_full = big.tile([128, KT, DV], F8, tag="v_full")
    nc.vector.memset(v_stg[:, :, D:DV], 1.0)

    # DRAM bounce buffers (Shared outputs are only supported for >4-core
    # groups, so these stay Local)
    bk_in = dram.tile([128, DC, MH], F8, tag="bk_in")
    bv_in = dram.tile([128, KH, DV], F8, tag="bv_in")
    bk_out_t = dram.tile([2, 128, DC, MH], F8, tag="bk_out")
    bv_out_t = dram.tile([2, 128, KH, DV], F8, tag="bv_out")
    bk_out = bk_out_t.opt()
    bv_out = bv_out_t.opt()

    pending = []

    def proj_ln(rbs, wt, x_ln):
        for mb, rb in enumerate(rbs):
            psum_ss = ps1.tile([128, BLK], F32, tag="vec")
            stg_t = stg.tile([128, DC, BLK], BF16, tag="stg")
            sq_pairs = [
                sq.tile([128, 2, BLK], F8, tag="sq", name=f"sqp{i}")
                for i in range(3)
            ]
            for nch in range(DC):
                psum_c = ps1.tile([128, DV], F32, tag="acc")
                for dc_ in range(DC):
                    nc.tensor.matmul(
                        psum_c[:, 0:BLK],
                        wt[:, nch, dc_, :],
                        rb[:, dc_, :],
                        start=(dc_ == 0),
                        stop=(dc_ == DC - 1),
                    )
                dst = stg_t[:, nch, :]
                nc.vector.tensor_copy(dst, psum_c[:, 0:BLK])
                nc.scalar.activation(
                    sq_pairs[nch // 2][:, nch % 2, :], dst, AF.Square
                )
            for cp in range(3):
                nc.tensor.matmul(
                    psum_ss, ones8, sq_pairs[cp],
                    start=(cp == 0), stop=(cp == 2), perf_mode=DR,
                )
            pending.append((psum_ss, stg_t, x_ln, mb))
            if len(pending) > 1:
                _finish(pending.pop(0))

    def _finish(pend):
        psum_ss, stg_t, x_ln, mb = pend
        rstd = rbc.tile([128, BLK], F32, tag="rbc")
        nc.scalar.activation(rstd, psum_ss, AF.Ln, scale=1.0 / D, bias=eps_t)
        nc.scalar.activation(rstd, rstd, AF.Exp, scale=-0.5)
        for nch in range(DC):
            dstv = x_ln[:, nch, mb * BLK : (mb + 1) * BLK]
            if apply_gb:
                nc.gpsimd.tensor_mul(stg_t[:, nch, :], stg_t[:, nch, :], rstd)
                nc.vector.tensor_scalar(
                    dstv,
                    stg_t[:, nch, :],
                    gm_sb[:, nch : nch + 1],
                    bt_sb[:, nch : nch + 1],
                    op0=mybir.AluOpType.mult,
                    op1=mybir.AluOpType.add,
                )
            elif nch % 2 == 0:
                # split the LN-scale writes across gpsimd and DVE so the
                # last block's finish is ~2x faster (k_stg gates the pair
                # exchange)
                nc.gpsimd.tensor_mul(dstv, stg_t[:, nch, :], rstd)
            else:
                nc.vector.tensor_mul(dstv, stg_t[:, nch, :], rstd)

    # ---- k projection (local key-half) + exchange ----
    proj_ln(k_rbs, wk_t, k_stg)
    while pending:
        _finish(pending.pop(0))
    nc.gpsimd.dma_start(bk_in, k_stg)
    nc.gpsimd.collective_compute(
        "AllGather",
        mybir.AluOpType.bypass,
        replica_groups=PAIRS,
        ins=[bk_in.opt()],
        outs=[bk_out],
    )
    nc.sync.dma_start(k_full[:, :, 0:MH], bk_out[0])
    nc.sync.dma_start(k_full[:, :, MH:LS], bk_out[1])

    # ---- v projection (local half, fp8 DoubleRow, token-major out) ----
    for mb, rb in enumerate(v_rbs):
        for mc in range(BLK // 128):
            tidx = mb * (BLK // 128) + mc
            psum_v = ps1.tile([128, DV], F32, tag="acc")
            for cp in range(DC // 2):
                lhsT = rb[:, 2 * cp : 2 * cp + 2, mc * 128 : (mc + 1) * 128]
                nc.tensor.matmul(
                    psum_v[:, 0:512], lhsT, wv_t[:, 2 * cp : 2 * cp + 2, 0:512],
                    start=(cp == 0), stop=(cp == DC // 2 - 1), perf_mode=DR,
                )
                nc.tensor.matmul(
                    psum_v[:, 512:768], lhsT,
                    wv_t[:, 2 * cp : 2 * cp + 2, 512:768],
                    start=(cp == 0), stop=(cp == DC // 2 - 1), perf_mode=DR,
                )
            # evacuations alternate DVE / scalar ACT-Copy so neither queue
            # saturates (the 16x host scaling of Wv folds out here)
            if tidx % 2 == 0:
                nc.vector.tensor_scalar_mul(
                    v_stg[:, tidx, 0:D], psum_v[:, 0:D], 1.0 / 16.0
                )
            else:
                nc.scalar.activation(
                    v_stg[:, tidx, 0:D], psum_v[:, 0:D], AF.Copy,
                    scale=1.0 / 16.0,
                )
    nc.gpsimd.dma_start(bv_in, v_stg)
    nc.gpsimd.collective_compute(
        "AllGather",
        mybir.AluOpType.bypass,
        replica_groups=PAIRS,
        ins=[bv_in.opt()],
        outs=[bv_out],
    )
    nc.sync.dma_start(v_full[:, 0:KH, :], bv_out[0])
    nc.sync.dma_start(v_full[:, KH:KT, :], bv_out[1])

    # ---- q projection (overlaps both gathers) ----
    proj_ln(q_rbs, wq_t, q_ln)
    while pending:
        _finish(pending.pop(0))

    ps1.release()
    ps2 = tc.alloc_tile_pool(name="ps2", bufs=2, space="PSUM")

    # ---- attention ----
    pts = []
    for qq in range(NQ):
        qs = qq * MQQ
        pt = ptp.tile([128, KT, MQQ], F8, tag="pt", name=f"pt{qq}")
        for tp in range(KT // 2):
            psum_s = ps2.tile([128, 2, MQQ], F32, tag="sc")
            for t2 in range(2):
                t = 2 * tp + t2
                for cp in range(DC // 2):
                    nc.tensor.matmul(
                        psum_s[:, t2, :],
                        k_full[:, 2 * cp : 2 * cp + 2, t * 128 : (t + 1) * 128],
                        q_ln[:, 2 * cp : 2 * cp + 2, qs : qs + MQQ],
                        start=(cp == 0),
                        stop=(cp == DC // 2 - 1),
                        perf_mode=DR,
                    )
            nc.scalar.activation(pt[:, 2 * tp : 2 * tp + 2, :], psum_s,
                                 AF.Exp, scale=SCALE, bias=ebias_t)
        pts.append(pt)
    for qq in range(NQ):
        qs = qq * MQQ
        pt = pts[qq]
        for mc in range(MQQ // 128):
            psum_o = ps2.tile([128, DV], F32, tag="bigp")
            for tp in range(KT // 2):
                lhsT = pt[:, 2 * tp : 2 * tp + 2, mc * 128 : (mc + 1) * 128]
                nc.tensor.matmul(
                    psum_o[:, 0:512], lhsT,
                    v_full[:, 2 * tp : 2 * tp + 2, 0:512],
                    start=(tp == 0), stop=(tp == KT // 2 - 1), perf_mode=DR,
                )
                nc.tensor.matmul(
                    psum_o[:, 512:DV], lhsT,
                    v_full[:, 2 * tp : 2 * tp + 2, 512:DV],
                    start=(tp == 0), stop=(tp == KT // 2 - 1), perf_mode=DR,
                )
            rse = scal.tile([128, 1], F32, tag="scal")
            nc.vector.reciprocal(rse, psum_o[:, D : D + 1])
            x = fin.tile([128, D], F32, tag="fin")
            sums = scal.tile([128, 1], F32, tag="scal")
            nc.scalar.activation(x, psum_o[:, 0:D], AF.Exp,
                                 scale=rse, accum_out=sums)
            rsum = scal.tile([128, 1], F32, tag="scal")
            nc.vector.reciprocal(rsum, sums)
            xh = fin.tile([128, D], F16, tag="xh")
            row = qs + mc * 128
            nc.vector.tensor_scalar_mul(xh[:, 0:384], x[:, 0:384], rsum)
            nc.sync.dma_start(out[row : row + 128, 0:384], xh[:, 0:384])
            nc.vector.tensor_scalar_mul(xh[:, 384:D], x[:, 384:D], rsum)
            nc.sync.dma_start(out[row : row + 128, 384:D], xh[:, 384:D])

    ps2.release()
    for p in reversed(pools.values()):
        p.release()


def _build(apply_gb):
    nc = bacc.Bacc(
        "TRN2", target_bir_lowering=False, debug=False, num_devices=N_CORES
    )
    qt = nc.dram_tensor("qt", [128, MQ // BLK, DC, BLK], BF16,
                        kind="ExternalInput").ap()
    kt = nc.dram_tensor("kt", [128, MH // BLK, DC, BLK], BF16,
                        kind="ExternalInput").ap()
    vt = nc.dram_tensor("vt", [128, MH // BLK, DC, BLK], F8,
                        kind="ExternalInput").ap()
    wq = nc.dram_tensor("wq", [DC, 128, DC, 128], BF16,
                        kind="ExternalInput").ap()
    wk = nc.dram_tensor("wk", [DC, 128, DC, 128], BF16,
                        kind="ExternalInput").ap()
    wv = nc.dram_tensor("wv", [128, DC, D], F8, kind="ExternalInput").ap()
    gm = nc.dram_tensor("gm", [D], F32, kind="ExternalInput").ap()
    bt = nc.dram_tensor("bt", [D], F32, kind="ExternalInput").ap()
    out = nc.dram_tensor("o", [MQ, D], F16, kind="ExternalOutput").ap()
    with tile.TileContext(nc) as tc:
        _emit(tc, (qt, kt, vt, wq, wk, wv, gm, bt), out, apply_gb)
    nc.compile()
    return nc


_NC_CACHE = {}


def _get_nc(apply_gb):
    if apply_gb not in _NC_CACHE:
        _NC_CACHE[apply_gb] = _build(apply_gb)
    return _NC_CACHE[apply_gb]


def _sparsify(x):
    b, l, d = x.shape
    return x.reshape(b, l // SEG, SEG, d)[:, :, ::RATE].reshape(b, -1, d)


def _pm(xT, dt):
    # [D, M] feature-major -> [128, M//BLK, DC, BLK] partition-major with
    # one contiguous run per (partition, block)
    m = xT.shape[1]
    return np.ascontiguousarray(
        xT.reshape(DC, 128, m // BLK, BLK).transpose(1, 2, 0, 3)
    ).astype(dt)


def _pw6(wT, dt):
    # [D_in, D_out] -> [DC_out, 128, DC_in, 128]: six output-column chunks,
    # each one contiguous 1536B run per partition on the SBUF side
    return np.ascontiguousarray(
        wT.reshape(DC, 128, DC, 128).transpose(2, 1, 0, 3)
    ).astype(dt)


def make_in_maps(Q, K, V, Wq, Wk, Wv, ln_gamma, ln_beta):
    npdt = mybir.dt.np(BF16)
    np8 = mybir.dt.np(F8)
    Qs = _sparsify(np.asarray(Q, dtype=np.float32))
    Ks = _sparsify(np.asarray(K, dtype=np.float32))
    Vs = _sparsify(np.asarray(V, dtype=np.float32))
    WqT = np.asarray(Wq, dtype=np.float32).T
    WkT = np.asarray(Wk, dtype=np.float32).T
    WvT = (np.asarray(Wv, dtype=np.float32).T * 16.0).astype(np8)
    WqTc = (WqT - WqT.mean(axis=1, keepdims=True)).astype(npdt)
    WkTc = (WkT - WkT.mean(axis=1, keepdims=True)).astype(npdt)
    gm = np.asarray(ln_gamma, dtype=np.float32)
    bt = np.asarray(ln_beta, dtype=np.float32)
    wq_a = _pw6(WqTc, npdt)
    wk_a = _pw6(WkTc, npdt)
    wv_a = np.ascontiguousarray(WvT.reshape(DC, 128, D).transpose(1, 0, 2))
    in_maps = []
    for c in range(N_CORES):
        b, h = c // 2, c % 2
        in_maps.append(
            {
                "qt": _pm(Qs[b, h * MQ : (h + 1) * MQ].T, npdt),
                "kt": _pm(Ks[b, h * MH : (h + 1) * MH].T, npdt),
                "vt": _pm(Vs[b, h * MH : (h + 1) * MH].T, np8),
                "wq": wq_a,
                "wk": wk_a,
                "wv": wv_a,
                "gm": gm,
                "bt": bt,
            }
        )
    return in_maps


def kernel(Q, K, V, Wq, Wk, Wv, ln_gamma, ln_beta, _run_kwargs=None):
    gm = np.asarray(ln_gamma, dtype=np.float32)
    bt = np.asarray(ln_beta, dtype=np.float32)
    apply_gb = not (np.all(gm == 1.0) and np.all(bt == 0.0))
    nc = _get_nc(apply_gb)
    in_maps = make_in_maps(Q, K, V, Wq, Wk, Wv, ln_gamma, ln_beta)
    try:
        res = run_bass_kernel_spmd(
            nc, in_maps, core_ids=list(range(N_CORES)), **(_run_kwargs or {})
        )
    except Exception:
        res = run_bass_kernel_spmd(
            nc, in_maps, core_ids=list(range(N_CORES)), **(_run_kwargs or {})
        )
    out = np.empty((B, LS, D), dtype=np.float32)
    for c in range(N_CORES):
        b, h = c // 2, c % 2
        out[b, h * MQ : (h + 1) * MQ, :] = res.results[c]["o"].astype(
            np.float32
        )
    if _run_kwargs:
        kernel.last_res = res
    return out
